# revision 4
# baseline (speedup 1.0000x reference)
"""ASP layer (low-rank masked attention + residual layernorm) on 8 TRN2 cores.

Sharding: core c handles batch b = c // 2, query half h = c % 2.
Each core receives x/mask for its batch ROTATED so that its 1024 queries are
rows 0:1024 (keys are just permuted, softmax+delta are invariant to key
order). The device program is identical on all cores (SPMD); only data
differs.

Device math per core (N=2048 keys, Q=1024 queries, D=1024, R=64):
  QtKt = [U|V]^T @ x^T             (PE, bf16, fp32 accum; x^T via DMA transpose)
  Qt   = QtKt[0:64]  * (mask*s).T  (DVE; s = 1/sqrt(r_eff) folded in on host)
  Kt   = QtKt[64:128] * mask.T     (DVE)
  S    = Qt_qblock^T @ Kt          (PE, per 128-query block; |S| small => no
                                    max subtraction needed before exp)
  A    = exp(S), rowsum via activation accum_out   (ACT, bf16 out)
  At   = PE transpose of A blocks  (bf16)
  delta= At^T @ x_bf               (PE, fp32 accum), scaled by 1/rowsum (ACT)
  out  = LN(x_q + delta)           (DVE bn_stats + ACT)  [gamma/beta on host]
"""

import sys

sys.path.insert(0, "/opt/trn_rl_repo")

import numpy as np
import ml_dtypes

B, N, D, R = 4, 2048, 1024, 64
NCORES = 8
Q = N // 2          # queries per core
NQB = Q // 128      # query blocks per core
NKT = N // 128      # key tiles
NDT = D // 128      # d tiles
LN_EPS = 1e-5

BF16 = ml_dtypes.bfloat16

_CACHE = {}


def _split_waits(nc, max_waits=1):
    """walrus in this container rejects instructions carrying more than ~1
    sem-wait (e.g. Drain/CTRL and the XPOSE DMA encodings). Move excess waits
    onto injected same-engine nops that precede the instruction — engines are
    program-ordered, so semantics are unchanged."""
    from concourse import mybir

    n = 0
    for fn in nc.m.functions:
        for bb in fn.blocks:
            insts = bb.instructions
            new_list = []
            for inst in insts:
                si = inst.sync_info
                waits = list(si.on_wait) if si and si.on_wait else []
                if len(waits) > max_waits:
                    excess = waits[: -max_waits]
                    si.on_wait = waits[-max_waits:]
                    for w in excess:
                        nop = mybir.InstNoOp(name=f"I-wsplit-{n}", ins=[],
                                             outs=[])
                        n += 1
                        nop.engine = inst.engine
                        nop.sync_info = mybir.SyncInfo(on_wait=[w],
                                                       on_update=[])
                        nc.register_instruction(nop)
                        new_list.append(nop)
                new_list.append(inst)
            insts[:] = new_list


def _patch_tile_drain():
    import concourse.tile as tile
    from concourse.vector_clock import ScopedClock

    if getattr(tile.TileContext, "_drain_patched", False):
        return

    def _drain_and_barrier(self, tick_clock, wait_clock):
        nc = self.nc
        drain_inst = nc.sync.drain()
        wait_clock.add_sem_waits(
            drain_inst.ins, ScopedClock({None: tick_clock.global_clock})
        )
        nc.all_engine_barrier()
        assert self.sems is not None
        popped = nc._tile_sem_poison_stack.pop()
        assert popped is self._sem_poison
        nc.clear_and_free_semaphores(list(self.sems.allocated().values()))
        nc.all_engine_barrier()
        _split_waits(nc)

    tile.TileContext._drain_and_barrier = _drain_and_barrier
    tile.TileContext._drain_patched = True


def build_program():
    import concourse.bass as bass
    import concourse.tile as tile
    from concourse import mybir

    _patch_tile_drain()
    f32 = mybir.dt.float32
    bf16 = mybir.dt.bfloat16
    AF = mybir.ActivationFunctionType

    nc = bass.Bass("TRN2", target_bir_lowering=False, debug=False,
                   num_devices=NCORES)

    xbf_d = nc.dram_tensor("xbf", [N, D], bf16, kind="ExternalInput").ap()
    xq_d = nc.dram_tensor("xq", [Q, D], f32, kind="ExternalInput").ap()
    mt_d = nc.dram_tensor("mt", [2 * R, N], f32, kind="ExternalInput").ap()
    uv_d = nc.dram_tensor("uv", [D, 2 * R], bf16, kind="ExternalInput").ap()
    id_d = nc.dram_tensor("ident", [128, 128], bf16, kind="ExternalInput").ap()
    out_d = nc.dram_tensor("out", [Q, D], f32, kind="ExternalOutput").ap()

    with tile.TileContext(nc) as tc:
        import contextlib
        with contextlib.ExitStack() as ctx:
            const = ctx.enter_context(tc.tile_pool(name="const", bufs=1))
            uv_sb = const.tile([128, NDT, 2 * R], bf16)
            nc.sync.dma_start(out=uv_sb,
                              in_=uv_d.rearrange("(t p) m -> p t m", p=128))
            mt_sb = const.tile([2 * R, N], f32)
            nc.sync.dma_start(out=mt_sb, in_=mt_d)
            id_sb = const.tile([128, 128], bf16)
            nc.sync.dma_start(out=id_sb, in_=id_d)
            eps_sb = const.tile([128, 1], f32)
            nc.vector.memset(eps_sb, LN_EPS)

            xbf_sb = const.tile([128, NKT, D], bf16)
            for kt in range(NKT):
                nc.sync.dma_start(out=xbf_sb[:, kt, :],
                                  in_=xbf_d[kt * 128:(kt + 1) * 128, :])
            xq_sb = const.tile([128, NQB, D], f32)
            for qb in range(NQB):
                nc.sync.dma_start(out=xq_sb[:, qb, :],
                                  in_=xq_d[qb * 128:(qb + 1) * 128, :])

            # x^T via bf16 DMA transpose, chunked over n for pipelining
            xT_sb = const.tile([128, NDT, N], bf16)
            for dt in range(NDT):
                for nch in range(4):
                    nc.sync.dma_start(
                        out=xT_sb[:, dt, nch * 512:(nch + 1) * 512],
                        in_=xbf_d[nch * 512:(nch + 1) * 512,
                                  dt * 128:(dt + 1) * 128],
                        transpose=True,
                    )

            qt_sb = const.tile([R, Q], bf16)
            kt_sb = const.tile([R, N], bf16)

            with tc.tile_pool(name="ps0", bufs=2, space="PSUM") as ps0:
                for nch in range(4):
                    lo, hi = nch * 512, (nch + 1) * 512
                    qk_ps = ps0.tile([128, 512], f32)
                    for dt in range(NDT):
                        nc.tensor.matmul(
                            qk_ps, uv_sb[:, dt, :],
                            xT_sb[:, dt, lo:hi],
                            start=(dt == 0), stop=(dt == NDT - 1),
                        )
                    if lo < Q:
                        nc.vector.tensor_mul(qt_sb[:, lo:hi],
                                             qk_ps[0:R, :], mt_sb[0:R, lo:hi])
                    nc.vector.tensor_mul(kt_sb[:, lo:hi],
                                         qk_ps[R:2 * R, :], mt_sb[R:2 * R, lo:hi])

            work = ctx.enter_context(tc.tile_pool(name="work", bufs=2))
            small = ctx.enter_context(tc.tile_pool(name="small", bufs=3))
            s_pool = ctx.enter_context(tc.tile_pool(name="s_ps", bufs=1,
                                                    space="PSUM"))
            at_pool = ctx.enter_context(tc.tile_pool(name="at_ps", bufs=2,
                                                     space="PSUM"))
            d_pool = ctx.enter_context(tc.tile_pool(name="d_ps", bufs=1,
                                                    space="PSUM"))

            for qb in range(NQB):
                qlo = qb * 128
                s_ps = s_pool.tile([128, N], f32)
                for c in range(4):
                    nc.tensor.matmul(
                        s_ps[:, c * 512:(c + 1) * 512],
                        qt_sb[:, qlo:qlo + 128],
                        kt_sb[:, c * 512:(c + 1) * 512],
                        start=True, stop=True,
                    )
                a_sb = work.tile([128, N], bf16, tag="a")
                rs = small.tile([128, 1], f32, tag="rs")
                nc.scalar.activation(out=a_sb, in_=s_ps, func=AF.Exp,
                                     accum_out=rs)
                rcp = small.tile([128, 1], f32, tag="rcp")
                nc.vector.reciprocal(rcp, rs)

                at_sb = work.tile([128, N], bf16, tag="at")
                for g in range(4):
                    at_ps = at_pool.tile([128, 512], bf16)
                    for j in range(4):
                        nc.tensor.transpose(
                            at_ps[:, j * 128:(j + 1) * 128],
                            a_sb[:, (4 * g + j) * 128:(4 * g + j + 1) * 128],
                            id_sb,
                        )
                    nc.scalar.copy(at_sb[:, g * 512:(g + 1) * 512], at_ps)

                d_ps = d_pool.tile([128, D], f32)
                for kt in range(NKT):
                    for dc in range(2):
                        nc.tensor.matmul(
                            d_ps[:, dc * 512:(dc + 1) * 512],
                            at_sb[:, kt * 128:(kt + 1) * 128],
                            xbf_sb[:, kt, dc * 512:(dc + 1) * 512],
                            start=(kt == 0), stop=(kt == NKT - 1),
                        )

                ds = work.tile([128, D], f32, tag="ds")
                nc.scalar.mul(ds, d_ps, rcp)
                y = work.tile([128, D], f32, tag="y")
                nc.vector.tensor_add(y, ds, xq_sb[:, qb, :])

                st6 = small.tile([128, 2, 6], f32, tag="st6")
                nc.vector.bn_stats(st6[:, 0, :], y[:, 0:512])
                nc.vector.bn_stats(st6[:, 1, :], y[:, 512:1024])
                mv = small.tile([128, 2], f32, tag="mv")
                nc.vector.bn_aggr(mv, st6)

                t_sb = work.tile([128, D], f32, tag="t")
                nc.vector.tensor_scalar_sub(t_sb, y, mv[:, 0:1])
                sd = small.tile([128, 1], f32, tag="sd")
                nc.scalar.activation(out=sd, in_=mv[:, 1:2], func=AF.Sqrt,
                                     bias=eps_sb)
                rstd = small.tile([128, 1], f32, tag="rstd")
                nc.vector.reciprocal(rstd, sd)
                o_sb = work.tile([128, D], f32, tag="o")
                nc.scalar.mul(o_sb, t_sb, rstd)
                nc.sync.dma_start(out=out_d[qlo:qlo + 128, :], in_=o_sb)

    return nc


def prep_core_inputs(x, mask, U, V):
    """Per-core input dicts (host-side sharding/layout prep)."""
    uv = np.concatenate([U, V], axis=1).astype(BF16)
    ident = np.eye(128, dtype=BF16)
    ins = []
    for c in range(NCORES):
        b, h = divmod(c, 2)
        rot = np.roll(np.arange(N), -h * Q)
        xr = np.ascontiguousarray(x[b][rot])            # [N, D] f32
        mr = np.ascontiguousarray(mask[b][rot])         # [N, R] f32
        s = 1.0 / np.sqrt(np.maximum(mr.sum(axis=1), 1.0))   # [N]
        mq = (mr * s[:, None]).T.astype(np.float32)     # [R, N]
        mk = mr.T.astype(np.float32)                    # [R, N]
        ins.append({
            "xbf": xr.astype(BF16),
            "xq": xr[:Q].astype(np.float32),
            "mt": np.ascontiguousarray(np.concatenate([mq, mk], axis=0)),
            "uv": uv,
            "ident": ident,
        })
    return ins


def run_cores(ins, trace=False, trace_kwargs=None):
    from concourse.bass_utils import run_bass_kernel_spmd

    if "nc" not in _CACHE:
        _CACHE["nc"] = build_program()
    kw = {}
    if trace:
        kw["trace"] = True
        kw.update(trace_kwargs or {})
    return run_bass_kernel_spmd(_CACHE["nc"], ins, list(range(NCORES)), **kw)


def kernel(x, mask, U, V, gamma, beta):
    x = np.asarray(x, dtype=np.float32)
    mask = np.asarray(mask, dtype=np.float32)
    U = np.asarray(U, dtype=np.float32)
    V = np.asarray(V, dtype=np.float32)
    gamma = np.asarray(gamma, dtype=np.float32)
    beta = np.asarray(beta, dtype=np.float32)

    ins = prep_core_inputs(x, mask, U, V)
    res = run_cores(ins)
    out = np.empty((B, N, D), dtype=np.float32)
    for c in range(NCORES):
        b, h = divmod(c, 2)
        out[b, h * Q:(h + 1) * Q] = res.results[c]["out"]
    return out * gamma + beta


# revision 5
# speedup vs baseline: 1.1840x; 1.1840x over previous
"""ASP layer (low-rank masked attention + residual layernorm) on 8 TRN2 cores.

Sharding: core c handles batch b = c // 2, query half h = c % 2.
Each core receives x/mask for its batch ROTATED so that its 1024 queries are
rows 0:1024 (keys are just permuted; softmax and delta are invariant to key
order). The device program is identical on all cores (SPMD); only data
differs.

Device math per core (N=2048 keys, Q=1024 queries, D=1024, R=64):
  QtKt = [U|V]^T @ x^T          (PE bf16, fp32 accum; x^T precomputed on host)
  Qt   = QtKt[0:64]   * (mask*s).T   (DVE; s = 1/sqrt(r_eff) folded on host)
  Kt   = QtKt[64:128] * mask.T       (DVE)
  St   = Kt_tile^T @ Qt         (PE; scores TRANSPOSED [k, q] so exp output
                                 is directly the delta stationary — no PE
                                 transposes of A needed. |S| is small so exp
                                 needs no max subtraction.)
  Et   = exp(St)                (ACT, psum -> sbuf bf16)
  rs   = ones^T @ Et            (PE M=1 matmul; softmax row sums)
  delta= Et^T @ x_bf            (PE bf16, fp32 accum; UNNORMALIZED)
  z    = rs*x_q + delta         (DVE; LayerNorm is scale-invariant per token,
                                 so LN(x + delta/rs) = LN(rs*x + delta))
  out  = LN(z)                  (DVE bn_stats; single batched sqrt on ACT to
                                 avoid Exp<->Sqrt activation-table thrash)
gamma/beta are applied on the host (they are tiny per-feature vectors).
"""

import sys

sys.path.insert(0, "/opt/trn_rl_repo")

import numpy as np
import ml_dtypes

B, N, D, R = 4, 2048, 1024, 64
NCORES = 8
Q = N // 2          # queries per core
NQB = Q // 128      # query blocks per core
NKT = N // 128      # key tiles
NDT = D // 128      # d tiles
LN_EPS = 1e-5
WARMUP_MM = 44      # ~4.4us of PE spin to lift the HAM clock gate early

BF16 = ml_dtypes.bfloat16

_CACHE = {}


def _split_waits(nc, max_waits=1):
    """walrus in this container rejects instructions carrying more than ~1
    sem-wait (e.g. Drain/CTRL and the XPOSE DMA encodings). Move excess waits
    onto injected same-engine nops that precede the instruction — engines are
    program-ordered, so semantics are unchanged."""
    from concourse import mybir

    n = 0
    for fn in nc.m.functions:
        for bb in fn.blocks:
            insts = bb.instructions
            new_list = []
            for inst in insts:
                si = inst.sync_info
                waits = list(si.on_wait) if si and si.on_wait else []
                if len(waits) > max_waits:
                    excess = waits[: -max_waits]
                    si.on_wait = waits[-max_waits:]
                    for w in excess:
                        nop = mybir.InstNoOp(name=f"I-wsplit-{n}", ins=[],
                                             outs=[])
                        n += 1
                        nop.engine = inst.engine
                        nop.sync_info = mybir.SyncInfo(on_wait=[w],
                                                       on_update=[])
                        nc.register_instruction(nop)
                        new_list.append(nop)
                new_list.append(inst)
            insts[:] = new_list


def _patch_tile_drain():
    import concourse.tile as tile
    from concourse.vector_clock import ScopedClock

    if getattr(tile.TileContext, "_drain_patched", False):
        return

    def _drain_and_barrier(self, tick_clock, wait_clock):
        nc = self.nc
        drain_inst = nc.sync.drain()
        wait_clock.add_sem_waits(
            drain_inst.ins, ScopedClock({None: tick_clock.global_clock})
        )
        nc.all_engine_barrier()
        assert self.sems is not None
        popped = nc._tile_sem_poison_stack.pop()
        assert popped is self._sem_poison
        nc.clear_and_free_semaphores(list(self.sems.allocated().values()))
        nc.all_engine_barrier()
        _split_waits(nc)

    tile.TileContext._drain_and_barrier = _drain_and_barrier
    tile.TileContext._drain_patched = True


def build_program():
    import contextlib

    import concourse.bass as bass
    import concourse.tile as tile
    from concourse import mybir

    _patch_tile_drain()
    f32 = mybir.dt.float32
    bf16 = mybir.dt.bfloat16
    AF = mybir.ActivationFunctionType

    nc = bass.Bass("TRN2", target_bir_lowering=False, debug=False,
                   num_devices=NCORES)

    xbf_d = nc.dram_tensor("xbf", [N, D], bf16, kind="ExternalInput").ap()
    xt_d = nc.dram_tensor("xt", [D, N], bf16, kind="ExternalInput").ap()
    xq_d = nc.dram_tensor("xq", [Q, D], f32, kind="ExternalInput").ap()
    mt_d = nc.dram_tensor("mt", [2 * R, N], f32, kind="ExternalInput").ap()
    uv_d = nc.dram_tensor("uv", [D, 2 * R], bf16, kind="ExternalInput").ap()
    id_d = nc.dram_tensor("ident", [128, 128], f32, kind="ExternalInput").ap()
    out_d = nc.dram_tensor("out", [Q, D], f32, kind="ExternalOutput").ap()

    with tile.TileContext(nc) as tc:
        with contextlib.ExitStack() as ctx:
            const = ctx.enter_context(tc.tile_pool(name="const", bufs=1))
            id_sb = const.tile([128, 128], f32)
            nc.sync.dma_start(out=id_sb, in_=id_d)
            eps_sb = const.tile([128, 1], f32)
            nc.vector.memset(eps_sb, LN_EPS)
            ones_sb = const.tile([128, 1], bf16)
            nc.vector.memset(ones_sb, 1.0)

            # PE warm-up spin: keeps TensorE busy ~4.4us so the HAM clock
            # gate opens before the real matmul stream begins.
            with tc.tile_pool(name="warm", bufs=1, space="PSUM") as warm:
                w_ps = warm.tile([128, 128], f32)
                for _ in range(WARMUP_MM):
                    nc.tensor.transpose(w_ps, id_sb, id_sb)

            uv_sb = const.tile([128, NDT, 2 * R], bf16)
            nc.sync.dma_start(out=uv_sb,
                              in_=uv_d.rearrange("(t p) m -> p t m", p=128))
            mt_sb = const.tile([2 * R, N], f32)
            nc.sync.dma_start(out=mt_sb, in_=mt_d)

            xt_sb = const.tile([128, NDT, N], bf16)
            for dt in range(NDT):
                nc.sync.dma_start(out=xt_sb[:, dt, :],
                                  in_=xt_d[dt * 128:(dt + 1) * 128, :])
            xbf_sb = const.tile([128, NKT, D], bf16)
            for kt in range(NKT):
                nc.sync.dma_start(out=xbf_sb[:, kt, :],
                                  in_=xbf_d[kt * 128:(kt + 1) * 128, :])

            qt_sb = const.tile([R, Q], bf16)
            kt_sb = const.tile([R, N], bf16)

            # ---- projections: QtKt = [U|V]^T @ x^T, then mask gates ----
            with tc.tile_pool(name="ps0", bufs=2, space="PSUM") as ps0:
                for nch in range(4):
                    lo, hi = nch * 512, (nch + 1) * 512
                    qk_ps = ps0.tile([128, 512], f32)
                    for dt in range(NDT):
                        nc.tensor.matmul(
                            qk_ps, uv_sb[:, dt, :],
                            xt_sb[:, dt, lo:hi],
                            start=(dt == 0), stop=(dt == NDT - 1),
                        )
                    if lo < Q:
                        nc.vector.tensor_mul(qt_sb[:, lo:hi],
                                             qk_ps[0:R, :], mt_sb[0:R, lo:hi])
                    nc.vector.tensor_mul(kt_sb[:, lo:hi],
                                         qk_ps[R:2 * R, :], mt_sb[R:2 * R, lo:hi])

            # ---- transposed scores + exp: Et[(kt,qc)] = exp(Kt_kt^T Qt_qc) ----
            et_sb = const.tile([128, 2, NKT, 512], bf16)
            st_pool = ctx.enter_context(
                tc.tile_pool(name="st_ps", bufs=2, space="PSUM"))
            rs_pool = ctx.enter_context(
                tc.tile_pool(name="rs_ps", bufs=1, space="PSUM"))
            rt_pool = ctx.enter_context(
                tc.tile_pool(name="rt_ps", bufs=1, space="PSUM"))
            d_pool = ctx.enter_context(
                tc.tile_pool(name="d_ps", bufs=2, space="PSUM"))
            work = ctx.enter_context(tc.tile_pool(name="work", bufs=2))
            keep = ctx.enter_context(tc.tile_pool(name="keep", bufs=1))
            small = ctx.enter_context(tc.tile_pool(name="small", bufs=3))

            for qc in range(2):
                qlo = qc * 512
                for kt in range(NKT):
                    st_ps = st_pool.tile([128, 512], f32)
                    nc.tensor.matmul(
                        st_ps,
                        kt_sb[:, kt * 128:(kt + 1) * 128],
                        qt_sb[:, qlo:qlo + 512],
                        start=True, stop=True,
                    )
                    nc.scalar.activation(out=et_sb[:, qc, kt, :], in_=st_ps,
                                         func=AF.Exp)

            # ---- softmax row sums: rs[1, q] = ones^T @ Et ----
            rsq_sb = keep.tile([128, NQB], f32)   # rs per query block, [q,1]
            for qc in range(2):
                rs_ps = rs_pool.tile([1, 512], f32)
                for kt in range(NKT):
                    nc.tensor.matmul(
                        rs_ps, ones_sb, et_sb[:, qc, kt, :],
                        start=(kt == 0), stop=(kt == NKT - 1),
                    )
                rs_sb = small.tile([1, 512], f32, tag="rs_sb")
                nc.vector.tensor_copy(rs_sb, rs_ps)
                for j in range(4):
                    qb = qc * 4 + j
                    rt_ps = rt_pool.tile([128, 1], f32)
                    nc.tensor.transpose(rt_ps, rs_sb[0:1, j * 128:(j + 1) * 128],
                                        id_sb[0:1, 0:1])
                    nc.vector.tensor_copy(rsq_sb[:, qb:qb + 1], rt_ps)

            # ---- delta + residual + LN stats per query block ----
            xq_pool = ctx.enter_context(tc.tile_pool(name="xq", bufs=2))
            tt_sb = const.tile([128, NQB, D], f32)   # centered y, kept to end
            var_sb = keep.tile([128, NQB], f32)
            for qb in range(NQB):
                qc, j = divmod(qb, 4)
                xq_sb = xq_pool.tile([128, D], f32)
                nc.sync.dma_start(out=xq_sb,
                                  in_=xq_d[qb * 128:(qb + 1) * 128, :])
                d_ps = d_pool.tile([128, D], f32)
                for kt in range(NKT):
                    for dc in range(2):
                        nc.tensor.matmul(
                            d_ps[:, dc * 512:(dc + 1) * 512],
                            et_sb[:, qc, kt, j * 128:(j + 1) * 128],
                            xbf_sb[:, kt, dc * 512:(dc + 1) * 512],
                            start=(kt == 0), stop=(kt == NKT - 1),
                        )
                y = work.tile([128, D], f32, tag="y")
                nc.vector.scalar_tensor_tensor(
                    out=y, in0=xq_sb, scalar=rsq_sb[:, qb:qb + 1], in1=d_ps,
                    op0=mybir.AluOpType.mult, op1=mybir.AluOpType.add,
                )
                st6 = small.tile([128, 2, 6], f32, tag="st6")
                nc.vector.bn_stats(st6[:, 0, :], y[:, 0:512])
                nc.vector.bn_stats(st6[:, 1, :], y[:, 512:1024])
                mv = small.tile([128, 2], f32, tag="mv")
                nc.vector.bn_aggr(mv, st6)
                nc.vector.tensor_scalar_sub(tt_sb[:, qb, :], y, mv[:, 0:1])
                nc.vector.tensor_copy(var_sb[:, qb:qb + 1], mv[:, 1:2])

            # ---- batched rstd (one Sqrt -> one ACT table load), then out ----
            sd_sb = keep.tile([128, NQB], f32)
            nc.scalar.activation(out=sd_sb, in_=var_sb, func=AF.Sqrt,
                                 bias=eps_sb)
            rstd_sb = keep.tile([128, NQB], f32)
            nc.vector.reciprocal(rstd_sb, sd_sb)
            for qb in range(NQB):
                o_sb = work.tile([128, D], f32, tag="o")
                nc.vector.tensor_scalar_mul(o_sb, tt_sb[:, qb, :],
                                            rstd_sb[:, qb:qb + 1])
                nc.sync.dma_start(out=out_d[qb * 128:(qb + 1) * 128, :],
                                  in_=o_sb)

    return nc


def prep_core_inputs(x, mask, U, V):
    """Per-core input dicts (host-side sharding/layout prep)."""
    uv = np.concatenate([U, V], axis=1).astype(BF16)
    ident = np.eye(128, dtype=np.float32)
    ins = []
    for c in range(NCORES):
        b, h = divmod(c, 2)
        rot = np.roll(np.arange(N), -h * Q)
        xr = np.ascontiguousarray(x[b][rot])            # [N, D] f32
        mr = np.ascontiguousarray(mask[b][rot])         # [N, R] f32
        s = 1.0 / np.sqrt(np.maximum(mr.sum(axis=1), 1.0))   # [N]
        mq = (mr * s[:, None]).T.astype(np.float32)     # [R, N]
        mk = mr.T.astype(np.float32)                    # [R, N]
        xbf = xr.astype(BF16)
        ins.append({
            "xbf": xbf,
            "xt": np.ascontiguousarray(xbf.T),
            "xq": xr[:Q].astype(np.float32),
            "mt": np.ascontiguousarray(np.concatenate([mq, mk], axis=0)),
            "uv": uv,
            "ident": ident,
        })
    return ins


def run_cores(ins, trace=False, trace_kwargs=None):
    from concourse.bass_utils import run_bass_kernel_spmd

    if "nc" not in _CACHE:
        _CACHE["nc"] = build_program()
    kw = {}
    if trace:
        kw["trace"] = True
        kw.update(trace_kwargs or {})
    return run_bass_kernel_spmd(_CACHE["nc"], ins, list(range(NCORES)), **kw)


def kernel(x, mask, U, V, gamma, beta):
    x = np.asarray(x, dtype=np.float32)
    mask = np.asarray(mask, dtype=np.float32)
    U = np.asarray(U, dtype=np.float32)
    V = np.asarray(V, dtype=np.float32)
    gamma = np.asarray(gamma, dtype=np.float32)
    beta = np.asarray(beta, dtype=np.float32)

    ins = prep_core_inputs(x, mask, U, V)
    res = run_cores(ins)
    out = np.empty((B, N, D), dtype=np.float32)
    for c in range(NCORES):
        b, h = divmod(c, 2)
        out[b, h * Q:(h + 1) * Q] = res.results[c]["out"]
    return out * gamma + beta


# revision 9
# speedup vs baseline: 1.2077x; 1.0200x over previous
"""ASP layer (low-rank masked attention + residual layernorm) on 8 TRN2 cores.

Sharding: core c handles batch b = c // 2, query half h = c % 2.
Each core receives x/mask for its batch ROTATED so that its 1024 queries are
rows 0:1024 (keys are just permuted; softmax and delta are invariant to key
order). The device program is identical on all cores (SPMD); only data
differs.

Device math per core (N=2048 keys, Q=1024 queries, D=1024, R=64):
  QtKt = [U|V]^T @ x^T          (PE bf16, fp32 accum; x^T precomputed on host)
  Qt   = QtKt[0:64]   * (mask*s).T   (DVE; s = 1/sqrt(r_eff) folded on host)
  Kt   = QtKt[64:128] * mask.T       (DVE)
  St   = Kt_tile^T @ Qt         (PE; scores TRANSPOSED [k, q] so exp output
                                 is directly the delta stationary — no PE
                                 transposes of A needed. |S| is small so exp
                                 needs no max subtraction.)
  Et   = exp(St)                (ACT, psum -> sbuf bf16)
  rs   = ones^T @ Et            (PE M=1 matmul; softmax row sums)
  delta= Et^T @ x_bf            (PE bf16, fp32 accum; UNNORMALIZED)
  z    = rs*x_q + delta         (DVE; LayerNorm is scale-invariant per token,
                                 so LN(x + delta/rs) = LN(rs*x + delta))
  out  = LN(z)                  (DVE bn_stats; single batched sqrt on ACT to
                                 avoid Exp<->Sqrt activation-table thrash)
gamma/beta are applied on the host (they are tiny per-feature vectors).
"""

import sys

sys.path.insert(0, "/opt/trn_rl_repo")

import numpy as np
import ml_dtypes

B, N, D, R = 4, 2048, 1024, 64
NCORES = 8
Q = N // 2          # queries per core
NQB = Q // 128      # query blocks per core
NKT = N // 128      # key tiles
NDT = D // 128      # d tiles
LN_EPS = 1e-5
WARMUP_MM = 70      # ~7us of PE spin to lift the HAM clock gate early

BF16 = ml_dtypes.bfloat16

_CACHE = {}


def _split_waits(nc, max_waits=1):
    """walrus in this container rejects instructions carrying more than ~1
    sem-wait (e.g. Drain/CTRL and the XPOSE DMA encodings). Move excess waits
    onto injected same-engine nops that precede the instruction — engines are
    program-ordered, so semantics are unchanged."""
    from concourse import mybir

    n = 0
    for fn in nc.m.functions:
        for bb in fn.blocks:
            insts = bb.instructions
            new_list = []
            for inst in insts:
                si = inst.sync_info
                waits = list(si.on_wait) if si and si.on_wait else []
                if len(waits) > max_waits:
                    excess = waits[: -max_waits]
                    si.on_wait = waits[-max_waits:]
                    for w in excess:
                        nop = mybir.InstNoOp(name=f"I-wsplit-{n}", ins=[],
                                             outs=[])
                        n += 1
                        nop.engine = inst.engine
                        nop.sync_info = mybir.SyncInfo(on_wait=[w],
                                                       on_update=[])
                        nc.register_instruction(nop)
                        new_list.append(nop)
                new_list.append(inst)
            insts[:] = new_list


def _patch_tile_drain():
    import concourse.tile as tile
    from concourse.vector_clock import ScopedClock

    if getattr(tile.TileContext, "_drain_patched", False):
        return

    def _drain_and_barrier(self, tick_clock, wait_clock):
        nc = self.nc
        drain_inst = nc.sync.drain()
        wait_clock.add_sem_waits(
            drain_inst.ins, ScopedClock({None: tick_clock.global_clock})
        )
        nc.all_engine_barrier()
        assert self.sems is not None
        popped = nc._tile_sem_poison_stack.pop()
        assert popped is self._sem_poison
        nc.clear_and_free_semaphores(list(self.sems.allocated().values()))
        nc.all_engine_barrier()
        _split_waits(nc)

    tile.TileContext._drain_and_barrier = _drain_and_barrier
    tile.TileContext._drain_patched = True


def build_program():
    import contextlib

    import concourse.bass as bass
    import concourse.tile as tile
    from concourse import mybir

    _patch_tile_drain()
    f32 = mybir.dt.float32
    bf16 = mybir.dt.bfloat16
    AF = mybir.ActivationFunctionType

    nc = bass.Bass("TRN2", target_bir_lowering=False, debug=False,
                   num_devices=NCORES)

    xbf_d = nc.dram_tensor("xbf", [N, D], bf16, kind="ExternalInput").ap()
    xt_d = nc.dram_tensor("xt", [D, N], bf16, kind="ExternalInput").ap()
    xq_d = nc.dram_tensor("xq", [Q, D], f32, kind="ExternalInput").ap()
    mt_d = nc.dram_tensor("mt", [2 * R, N], f32, kind="ExternalInput").ap()
    uv_d = nc.dram_tensor("uv", [D, 2 * R], bf16, kind="ExternalInput").ap()
    id_d = nc.dram_tensor("ident", [128, 128], f32, kind="ExternalInput").ap()
    out_d = nc.dram_tensor("out", [Q, D], f32, kind="ExternalOutput").ap()

    with tile.TileContext(nc) as tc:
        with contextlib.ExitStack() as ctx:
            const = ctx.enter_context(tc.tile_pool(name="const", bufs=1))
            id_sb = const.tile([128, 128], f32)
            nc.sync.dma_start(out=id_sb, in_=id_d)
            eps_sb = const.tile([128, 1], f32)
            nc.vector.memset(eps_sb, LN_EPS)
            ones_sb = const.tile([128, 1], bf16)
            nc.vector.memset(ones_sb, 1.0)
            warm_sb = const.tile([128, 128], bf16)
            nc.vector.memset(warm_sb, 0.5)

            # PE warm-up spin with NORMAL-mode matmuls (transpose-mode is
            # invisible to the HAM activity monitor): keeps TensorE busy so
            # the clock gate opens before the real matmul stream begins.
            with tc.tile_pool(name="warm", bufs=1, space="PSUM") as warm:
                w_ps = warm.tile([128, 128], f32)
                for _ in range(WARMUP_MM):
                    nc.tensor.matmul(w_ps, warm_sb, warm_sb,
                                     start=True, stop=True)

            uv_sb = const.tile([128, NDT, 2 * R], bf16)
            nc.sync.dma_start(out=uv_sb,
                              in_=uv_d.rearrange("(t p) m -> p t m", p=128))
            mt_sb = const.tile([2 * R, N], f32)
            nc.sync.dma_start(out=mt_sb, in_=mt_d)

            xt_sb = const.tile([128, NDT, N], bf16)
            for dt in range(NDT):
                nc.sync.dma_start(out=xt_sb[:, dt, :],
                                  in_=xt_d[dt * 128:(dt + 1) * 128, :])
            xbf_sb = const.tile([128, NKT, D], bf16)
            for kt in range(NKT):
                nc.sync.dma_start(out=xbf_sb[:, kt, :],
                                  in_=xbf_d[kt * 128:(kt + 1) * 128, :])

            qt_sb = const.tile([R, Q], bf16)
            kt_sb = const.tile([R, N], bf16)
            # Et layout: [p, qc, t(=kt pair), 1024] so one exp covers 2 kt
            et_sb = const.tile([128, 2, NKT // 2, 1024], bf16)
            work = ctx.enter_context(tc.tile_pool(name="work", bufs=2))
            keep = ctx.enter_context(tc.tile_pool(name="keep", bufs=1))
            small = ctx.enter_context(tc.tile_pool(name="small", bufs=3))
            rsq_sb = keep.tile([128, NQB], f32)   # softmax rowsums, [q,1]/qb

            # ---- projections: QtKt = [U|V]^T @ x^T, then mask gates ----
            with tc.tile_pool(name="ps0", bufs=2, space="PSUM") as ps0:
                for nch in range(4):
                    lo, hi = nch * 512, (nch + 1) * 512
                    qk_ps = ps0.tile([128, 512], f32)
                    for dt in range(NDT):
                        nc.tensor.matmul(
                            qk_ps, uv_sb[:, dt, :],
                            xt_sb[:, dt, lo:hi],
                            start=(dt == 0), stop=(dt == NDT - 1),
                        )
                    if lo < Q:
                        nc.vector.tensor_mul(qt_sb[:, lo:hi],
                                             qk_ps[0:R, :], mt_sb[0:R, lo:hi])
                    nc.vector.tensor_mul(kt_sb[:, lo:hi],
                                         qk_ps[R:2 * R, :],
                                         mt_sb[R:2 * R, lo:hi])

            with contextlib.ExitStack() as phase_a:
                # ---- transposed scores + exp + rowsums ----
                # St[(kt,qc)] = Kt_kt^T @ Qt_qc ; Et = exp(St) ;
                # rs[1,q] = ones^T @ Et (PE M=1 matmul, accumulated over kt)
                st_pool = phase_a.enter_context(
                    tc.tile_pool(name="st_ps", bufs=2, space="PSUM"))
                rs_pool = phase_a.enter_context(
                    tc.tile_pool(name="rs_ps", bufs=1, space="PSUM"))
                rt_pool = phase_a.enter_context(
                    tc.tile_pool(name="rt_ps", bufs=1, space="PSUM"))
                rs_ps = {}
                for qc in range(2):
                    qlo = qc * 512
                    rs_tile = rs_pool.tile([1, 512], f32, tag=f"rs{qc}")
                    rs_ps[qc] = rs_tile
                    for t in range(NKT // 2):
                        st_ps = st_pool.tile([128, 1024], f32)
                        for h in range(2):
                            kt = 2 * t + h
                            nc.tensor.matmul(
                                st_ps[:, h * 512:(h + 1) * 512],
                                kt_sb[:, kt * 128:(kt + 1) * 128],
                                qt_sb[:, qlo:qlo + 512],
                                start=True, stop=True,
                            )
                        nc.scalar.activation(out=et_sb[:, qc, t, :],
                                             in_=st_ps, func=AF.Exp)
                        for h in range(2):
                            kt = 2 * t + h
                            nc.tensor.matmul(
                                rs_ps[qc], ones_sb,
                                et_sb[:, qc, t, h * 512:(h + 1) * 512],
                                start=(kt == 0), stop=(kt == NKT - 1),
                            )
                # rowsum layout fix: [1, q] -> [q, 1] per query block via a
                # tiny PE transpose (out = rs_slice^T @ id[0:1,0:1])
                for qc in range(2):
                    rs_sb = small.tile([1, 512], f32, tag="rs_sb")
                    nc.vector.tensor_copy(rs_sb, rs_ps[qc])
                    for j in range(4):
                        qb = qc * 4 + j
                        rt_ps = rt_pool.tile([128, 1], f32)
                        nc.tensor.transpose(rt_ps,
                                            rs_sb[0:1, j * 128:(j + 1) * 128],
                                            id_sb[0:1, 0:1])
                        nc.vector.tensor_copy(rsq_sb[:, qb:qb + 1], rt_ps)

            # ---- delta + residual + LN per query block ----
            d_pool = ctx.enter_context(
                tc.tile_pool(name="d_ps", bufs=2, space="PSUM"))
            xq_pool = ctx.enter_context(tc.tile_pool(name="xq", bufs=2))
            for qb in range(NQB):
                qc, j = divmod(qb, 4)
                xq_sb = xq_pool.tile([128, D], f32)
                nc.sync.dma_start(out=xq_sb,
                                  in_=xq_d[qb * 128:(qb + 1) * 128, :])
                d_ps = d_pool.tile([128, D], f32)
                for kt in range(NKT):
                    t, h = divmod(kt, 2)
                    for dc in range(2):
                        nc.tensor.matmul(
                            d_ps[:, dc * 512:(dc + 1) * 512],
                            et_sb[:, qc, t, h * 512 + j * 128:
                                  h * 512 + (j + 1) * 128],
                            xbf_sb[:, kt, dc * 512:(dc + 1) * 512],
                            start=(kt == 0), stop=(kt == NKT - 1),
                        )
                # z = rs*x_q + delta ; LN(z) == LN(x_q + delta/rs)
                y = work.tile([128, D], f32, tag="y")
                nc.vector.scalar_tensor_tensor(
                    out=y, in0=xq_sb, scalar=rsq_sb[:, qb:qb + 1], in1=d_ps,
                    op0=mybir.AluOpType.mult, op1=mybir.AluOpType.add,
                )
                st6 = small.tile([128, 2, 6], f32, tag="st6")
                nc.vector.bn_stats(st6[:, 0, :], y[:, 0:512])
                nc.vector.bn_stats(st6[:, 1, :], y[:, 512:1024])
                mv = small.tile([128, 2], f32, tag="mv")
                nc.vector.bn_aggr(mv, st6)
                t_sb = work.tile([128, D], f32, tag="t")
                nc.vector.tensor_scalar_sub(t_sb, y, mv[:, 0:1])
                sd = small.tile([128, 1], f32, tag="sd")
                nc.scalar.activation(out=sd, in_=mv[:, 1:2], func=AF.Sqrt,
                                     bias=eps_sb)
                rstd = small.tile([128, 1], f32, tag="rstd")
                nc.vector.reciprocal(rstd, sd)
                o_sb = work.tile([128, D], f32, tag="o")
                nc.vector.tensor_scalar_mul(o_sb, t_sb, rstd)
                nc.sync.dma_start(out=out_d[qb * 128:(qb + 1) * 128, :],
                                  in_=o_sb)

    return nc


def prep_core_inputs(x, mask, U, V):
    """Per-core input dicts (host-side sharding/layout prep)."""
    uv = np.concatenate([U, V], axis=1).astype(BF16)
    ident = np.eye(128, dtype=np.float32)
    ins = []
    for c in range(NCORES):
        b, h = divmod(c, 2)
        rot = np.roll(np.arange(N), -h * Q)
        xr = np.ascontiguousarray(x[b][rot])            # [N, D] f32
        mr = np.ascontiguousarray(mask[b][rot])         # [N, R] f32
        s = 1.0 / np.sqrt(np.maximum(mr.sum(axis=1), 1.0))   # [N]
        mq = (mr * s[:, None]).T.astype(np.float32)     # [R, N]
        mk = mr.T.astype(np.float32)                    # [R, N]
        xbf = xr.astype(BF16)
        ins.append({
            "xbf": xbf,
            "xt": np.ascontiguousarray(xbf.T),
            "xq": xr[:Q].astype(np.float32),
            "mt": np.ascontiguousarray(np.concatenate([mq, mk], axis=0)),
            "uv": uv,
            "ident": ident,
        })
    return ins


def run_cores(ins, trace=False, trace_kwargs=None):
    from concourse.bass_utils import run_bass_kernel_spmd

    if "nc" not in _CACHE:
        _CACHE["nc"] = build_program()
    kw = {}
    if trace:
        kw["trace"] = True
        kw.update(trace_kwargs or {})
    return run_bass_kernel_spmd(_CACHE["nc"], ins, list(range(NCORES)), **kw)


def kernel(x, mask, U, V, gamma, beta):
    x = np.asarray(x, dtype=np.float32)
    mask = np.asarray(mask, dtype=np.float32)
    U = np.asarray(U, dtype=np.float32)
    V = np.asarray(V, dtype=np.float32)
    gamma = np.asarray(gamma, dtype=np.float32)
    beta = np.asarray(beta, dtype=np.float32)

    ins = prep_core_inputs(x, mask, U, V)
    res = run_cores(ins)
    out = np.empty((B, N, D), dtype=np.float32)
    for c in range(NCORES):
        b, h = divmod(c, 2)
        out[b, h * Q:(h + 1) * Q] = res.results[c]["out"]
    return out * gamma + beta


# revision 13
# speedup vs baseline: 1.6499x; 1.3661x over previous
"""ASP layer (low-rank masked attention + residual layernorm) on 8 TRN2 cores.

Sharding: core c handles batch b = c // 2, query half h = c % 2.
Each core receives x/mask for its batch ROTATED so that its 1024 queries are
rows 0:1024 (keys are just permuted; softmax and delta are invariant to key
order). The device program is identical on all cores (SPMD); only data
differs.

Device math per core (N=2048 keys, Q=1024 queries, D=1024, R=64):
  QtKt = [U|V]^T @ x^T          (PE bf16, fp32 accum; x^T precomputed on host)
  Qt   = QtKt[0:64]   * (mask*s).T   (DVE; s = 1/sqrt(r_eff) folded on host)
  Kt   = QtKt[64:128] * mask.T       (DVE)
  St   = Kt_tile^T @ Qt         (PE; scores TRANSPOSED [k, q] so exp output
                                 is directly the delta stationary — no PE
                                 transposes of A needed. |S| is small so exp
                                 needs no max subtraction.)
  Et   = exp(St)                (ACT, psum -> sbuf bf16)
  rs   = ones^T @ Et            (PE M=1 matmul; softmax row sums)
  delta= Et^T @ x_bf            (PE bf16, fp32 accum; UNNORMALIZED)
  z    = rs*x_q + delta         (DVE; LayerNorm is scale-invariant per token,
                                 so LN(x + delta/rs) = LN(rs*x + delta))
  out  = LN(z)                  (DVE bn_stats; single batched sqrt on ACT to
                                 avoid Exp<->Sqrt activation-table thrash)
gamma/beta are applied on the host (they are tiny per-feature vectors).
"""

import sys

sys.path.insert(0, "/opt/trn_rl_repo")

import numpy as np
import ml_dtypes

B, N, D, R = 4, 2048, 1024, 64
NCORES = 8
Q = N // 2          # queries per core
NQB = Q // 128      # query blocks per core
NKT = N // 128      # key tiles
NDT = D // 128      # d tiles
LN_EPS = 1e-5
WARMUP_MM = 70      # ~7us of PE spin to lift the HAM clock gate early

BF16 = ml_dtypes.bfloat16

_CACHE = {}


def _split_waits(nc, max_waits=1):
    """walrus in this container rejects instructions carrying more than ~1
    sem-wait (e.g. Drain/CTRL and the XPOSE DMA encodings). Move excess waits
    onto injected same-engine nops that precede the instruction — engines are
    program-ordered, so semantics are unchanged."""
    from concourse import mybir

    n = 0
    for fn in nc.m.functions:
        for bb in fn.blocks:
            insts = bb.instructions
            new_list = []
            for inst in insts:
                si = inst.sync_info
                waits = list(si.on_wait) if si and si.on_wait else []
                if len(waits) > max_waits:
                    excess = waits[: -max_waits]
                    si.on_wait = waits[-max_waits:]
                    for w in excess:
                        nop = mybir.InstNoOp(name=f"I-wsplit-{n}", ins=[],
                                             outs=[])
                        n += 1
                        nop.engine = inst.engine
                        nop.sync_info = mybir.SyncInfo(on_wait=[w],
                                                       on_update=[])
                        nc.register_instruction(nop)
                        new_list.append(nop)
                new_list.append(inst)
            insts[:] = new_list


def _patch_tile_drain():
    import concourse.tile as tile
    from concourse.vector_clock import ScopedClock

    if getattr(tile.TileContext, "_drain_patched", False):
        return

    def _drain_and_barrier(self, tick_clock, wait_clock):
        nc = self.nc
        drain_inst = nc.sync.drain()
        wait_clock.add_sem_waits(
            drain_inst.ins, ScopedClock({None: tick_clock.global_clock})
        )
        nc.all_engine_barrier()
        assert self.sems is not None
        popped = nc._tile_sem_poison_stack.pop()
        assert popped is self._sem_poison
        nc.clear_and_free_semaphores(list(self.sems.allocated().values()))
        nc.all_engine_barrier()
        _split_waits(nc)

    tile.TileContext._drain_and_barrier = _drain_and_barrier
    tile.TileContext._drain_patched = True


def build_program():
    import contextlib

    import concourse.bass as bass
    import concourse.tile as tile
    from concourse import mybir

    _patch_tile_drain()
    f32 = mybir.dt.float32
    bf16 = mybir.dt.bfloat16
    AF = mybir.ActivationFunctionType

    nc = bass.Bass("TRN2", target_bir_lowering=False, debug=False,
                   num_devices=NCORES)

    xbf_d = nc.dram_tensor("xbf", [N, D], bf16, kind="ExternalInput").ap()
    xt_d = nc.dram_tensor("xt", [D, N], bf16, kind="ExternalInput").ap()
    xq_d = nc.dram_tensor("xq", [Q, D], f32, kind="ExternalInput").ap()
    mt_d = nc.dram_tensor("mt", [2 * R, N], f32, kind="ExternalInput").ap()
    uv_d = nc.dram_tensor("uv", [D, 2 * R], bf16, kind="ExternalInput").ap()
    id_d = nc.dram_tensor("ident", [128, 128], f32, kind="ExternalInput").ap()
    out_d = nc.dram_tensor("out", [Q, D], f32, kind="ExternalOutput").ap()

    with tile.TileContext(nc) as tc:
        with contextlib.ExitStack() as ctx:
            const = ctx.enter_context(tc.tile_pool(name="const", bufs=1))
            id_sb = const.tile([128, 128], f32)
            nc.sync.dma_start(out=id_sb, in_=id_d)
            eps_sb = const.tile([128, 1], f32)
            nc.vector.memset(eps_sb, LN_EPS)
            ones_sb = const.tile([128, 1], bf16)
            nc.vector.memset(ones_sb, 1.0)
            warm_sb = const.tile([128, 128], bf16)
            nc.vector.memset(warm_sb, 0.5)

            # PE warm-up spin with NORMAL-mode matmuls (transpose-mode is
            # invisible to the HAM activity monitor): keeps TensorE busy so
            # the clock gate opens before the real matmul stream begins.
            with tc.tile_pool(name="warm", bufs=1, space="PSUM") as warm:
                w_ps = warm.tile([128, 128], f32)
                for _ in range(WARMUP_MM):
                    nc.tensor.matmul(w_ps, warm_sb, warm_sb,
                                     start=True, stop=True)

            uv_sb = const.tile([128, NDT, 2 * R], bf16)
            nc.sync.dma_start(out=uv_sb,
                              in_=uv_d.rearrange("(t p) m -> p t m", p=128))
            mt_sb = const.tile([2 * R, N], f32)
            nc.sync.dma_start(out=mt_sb, in_=mt_d)

            # x^T loaded chunk-major so projection chunk nch only waits on
            # its own 8 sub-loads, not the whole 4MB
            xt_sb = const.tile([128, NDT, N], bf16)
            for nch in range(4):
                for dt in range(NDT):
                    nc.sync.dma_start(
                        out=xt_sb[:, dt, nch * 512:(nch + 1) * 512],
                        in_=xt_d[dt * 128:(dt + 1) * 128,
                                 nch * 512:(nch + 1) * 512])
            xbf_sb = const.tile([128, NKT, D], bf16)
            for kt in range(NKT):
                nc.sync.dma_start(out=xbf_sb[:, kt, :],
                                  in_=xbf_d[kt * 128:(kt + 1) * 128, :])

            qt_sb = const.tile([R, Q], bf16)
            kt_sb = const.tile([R, N], bf16)
            # Et layout: [p, qc, t(=kt pair), 1024] so one exp covers 2 kt
            et_sb = const.tile([128, 2, NKT // 2, 1024], bf16)
            work = ctx.enter_context(tc.tile_pool(name="work", bufs=2))
            keep = ctx.enter_context(tc.tile_pool(name="keep", bufs=1))
            small = ctx.enter_context(tc.tile_pool(name="small", bufs=3))
            rsq_sb = keep.tile([128, NQB], f32)   # softmax rowsums, [q,1]/qb

            st_pool = ctx.enter_context(
                tc.tile_pool(name="st_ps", bufs=2, space="PSUM"))
            xq_pool = ctx.enter_context(tc.tile_pool(name="xq", bufs=2))
            P = {}  # rr/d PSUM pools open after ps0 closes (bank budget)

            def st_pair(qc, t):
                """St = Kt_kt^T @ Qt_qc for kt pair (2t, 2t+1); Et = exp."""
                qlo = qc * 512
                st_ps = st_pool.tile([128, 1024], f32, name=f"st_{qc}_{t}", tag="st")
                for h in range(2):
                    kt = 2 * t + h
                    nc.tensor.matmul(
                        st_ps[:, h * 512:(h + 1) * 512],
                        kt_sb[:, kt * 128:(kt + 1) * 128],
                        qt_sb[:, qlo:qlo + 512],
                        start=True, stop=True,
                    )
                nc.scalar.activation(out=et_sb[:, qc, t, :], in_=st_ps,
                                     func=AF.Exp)

            def rowsums(qc):
                """rs[1,q] = ones^T @ Et (M=1 matmul over kt), then fix the
                layout [1,q] -> [q,1] per query block with a tiny PE
                transpose. One shared PSUM bank hosts both outputs."""
                rr_ps = P["rr"].tile([128, 512], f32, name=f"rr_{qc}",
                                     tag="rr")
                for kt in range(NKT):
                    t, h = divmod(kt, 2)
                    nc.tensor.matmul(
                        rr_ps[0:1, :], ones_sb,
                        et_sb[:, qc, t, h * 512:(h + 1) * 512],
                        start=(kt == 0), stop=(kt == NKT - 1),
                    )
                rs_sb = small.tile([1, 512], f32, tag="rs_sb")
                nc.vector.tensor_copy(rs_sb, rr_ps[0:1, :])
                for j in range(4):
                    qb = qc * 4 + j
                    nc.tensor.transpose(rr_ps[:, j:j + 1],
                                        rs_sb[0:1, j * 128:(j + 1) * 128],
                                        id_sb[0:1, 0:1])
                    nc.vector.tensor_copy(rsq_sb[:, qb:qb + 1],
                                          rr_ps[:, j:j + 1])

            def delta_mms(qb):
                """Unnormalized delta = Et^T @ x_bf, as a list of thunks so
                the caller can interleave them with other PE work."""
                qc, j = divmod(qb, 4)
                xq_sb = xq_pool.tile([128, D], f32, name=f"xq_{qb}", tag="xq")
                nc.sync.dma_start(out=xq_sb,
                                  in_=xq_d[qb * 128:(qb + 1) * 128, :])
                d_ps = [P["d"].tile([128, 512], f32, name=f"d_{qb}_{dc}",
                                     tag="d") for dc in range(2)]
                thunks = []
                for dc in range(2):
                    for kt in range(NKT):
                        t, h = divmod(kt, 2)
                        thunks.append(lambda dc=dc, kt=kt, t=t, h=h: (
                            nc.tensor.matmul(
                                d_ps[dc],
                                et_sb[:, qc, t, h * 512 + j * 128:
                                      h * 512 + (j + 1) * 128],
                                xbf_sb[:, kt, dc * 512:(dc + 1) * 512],
                                start=(kt == 0), stop=(kt == NKT - 1),
                            )))
                return xq_sb, d_ps, thunks

            def epilogue(qb, xq_sb, d_ps):
                """z = rs*x_q + delta; out = LN(z) (== LN(x_q + delta/rs))."""
                y = work.tile([128, D], f32, tag="y")
                for dc in range(2):
                    lo, hi = dc * 512, (dc + 1) * 512
                    nc.vector.scalar_tensor_tensor(
                        out=y[:, lo:hi], in0=xq_sb[:, lo:hi],
                        scalar=rsq_sb[:, qb:qb + 1], in1=d_ps[dc],
                        op0=mybir.AluOpType.mult, op1=mybir.AluOpType.add,
                    )
                st6 = small.tile([128, 2, 6], f32, tag="st6")
                nc.vector.bn_stats(st6[:, 0, :], y[:, 0:512])
                nc.vector.bn_stats(st6[:, 1, :], y[:, 512:1024])
                mv = small.tile([128, 2], f32, tag="mv")
                nc.vector.bn_aggr(mv, st6)
                t_sb = work.tile([128, D], f32, tag="t")
                nc.vector.tensor_scalar_sub(t_sb, y, mv[:, 0:1])
                sd = small.tile([128, 1], f32, tag="sd")
                nc.scalar.activation(out=sd, in_=mv[:, 1:2], func=AF.Sqrt,
                                     bias=eps_sb)
                rstd = small.tile([128, 1], f32, tag="rstd")
                nc.vector.reciprocal(rstd, sd)
                o_sb = work.tile([128, D], f32, tag="o")
                nc.vector.tensor_scalar_mul(o_sb, t_sb, rstd)
                nc.sync.dma_start(out=out_d[qb * 128:(qb + 1) * 128, :],
                                  in_=o_sb)

            # ---- phase 0/A: projections interleaved with qc0 scores so
            # the PE stream stays dense while exps run on ACT ----
            with tc.tile_pool(name="ps0", bufs=2, space="PSUM") as ps0:
                for nch in range(4):
                    lo, hi = nch * 512, (nch + 1) * 512
                    qk_ps = ps0.tile([128, 512], f32)
                    for dt in range(NDT):
                        nc.tensor.matmul(
                            qk_ps, uv_sb[:, dt, :],
                            xt_sb[:, dt, lo:hi],
                            start=(dt == 0), stop=(dt == NDT - 1),
                        )
                    if lo < Q:
                        nc.vector.tensor_mul(qt_sb[:, lo:hi],
                                             qk_ps[0:R, :], mt_sb[0:R, lo:hi])
                    nc.vector.tensor_mul(kt_sb[:, lo:hi],
                                         qk_ps[R:2 * R, :],
                                         mt_sb[R:2 * R, lo:hi])
                    if nch >= 1:
                        # Kt tiles 0..4*nch-1 and Qt[0:512] are ready
                        st_pair(0, 2 * (nch - 1))
                        st_pair(0, 2 * (nch - 1) + 1)
                st_pair(0, 6)
                st_pair(0, 7)

            P["rr"] = ctx.enter_context(
                tc.tile_pool(name="rr_ps", bufs=1, space="PSUM"))
            P["d"] = ctx.enter_context(
                tc.tile_pool(name="d_ps", bufs=3, space="PSUM"))
            rowsums(0)

            # ---- qc1 scores interleaved with qb0/qb1 deltas ----
            xq0, dps0, th0 = delta_mms(0)
            xq1, dps1, th1 = delta_mms(1)
            th01 = th0 + th1
            for t in range(NKT // 2):
                st_pair(1, t)
                for mm in th01[t * 8:(t + 1) * 8]:
                    mm()
            epilogue(0, xq0, dps0)
            epilogue(1, xq1, dps1)

            rowsums(1)

            for qb in range(2, NQB):
                xq_sb, d_ps, thunks = delta_mms(qb)
                for mm in thunks:
                    mm()
                epilogue(qb, xq_sb, d_ps)

    return nc


def prep_core_inputs(x, mask, U, V):
    """Per-core input dicts (host-side sharding/layout prep)."""
    uv = np.concatenate([U, V], axis=1).astype(BF16)
    ident = np.eye(128, dtype=np.float32)
    ins = []
    for c in range(NCORES):
        b, h = divmod(c, 2)
        rot = np.roll(np.arange(N), -h * Q)
        xr = np.ascontiguousarray(x[b][rot])            # [N, D] f32
        mr = np.ascontiguousarray(mask[b][rot])         # [N, R] f32
        s = 1.0 / np.sqrt(np.maximum(mr.sum(axis=1), 1.0))   # [N]
        mq = (mr * s[:, None]).T.astype(np.float32)     # [R, N]
        mk = mr.T.astype(np.float32)                    # [R, N]
        xbf = xr.astype(BF16)
        ins.append({
            "xbf": xbf,
            "xt": np.ascontiguousarray(xbf.T),
            "xq": xr[:Q].astype(np.float32),
            "mt": np.ascontiguousarray(np.concatenate([mq, mk], axis=0)),
            "uv": uv,
            "ident": ident,
        })
    return ins


def run_cores(ins, trace=False, trace_kwargs=None):
    from concourse.bass_utils import run_bass_kernel_spmd

    if "nc" not in _CACHE:
        _CACHE["nc"] = build_program()
    kw = {}
    if trace:
        kw["trace"] = True
        kw.update(trace_kwargs or {})
    return run_bass_kernel_spmd(_CACHE["nc"], ins, list(range(NCORES)), **kw)


def kernel(x, mask, U, V, gamma, beta):
    x = np.asarray(x, dtype=np.float32)
    mask = np.asarray(mask, dtype=np.float32)
    U = np.asarray(U, dtype=np.float32)
    V = np.asarray(V, dtype=np.float32)
    gamma = np.asarray(gamma, dtype=np.float32)
    beta = np.asarray(beta, dtype=np.float32)

    ins = prep_core_inputs(x, mask, U, V)
    res = run_cores(ins)
    out = np.empty((B, N, D), dtype=np.float32)
    for c in range(NCORES):
        b, h = divmod(c, 2)
        out[b, h * Q:(h + 1) * Q] = res.results[c]["out"]
    return out * gamma + beta
